# revision 20
# baseline (speedup 1.0000x reference)
"""Memory-efficient Gaussian rasterizer on 8 Trainium2 NeuronCores.

Strategy (tile-parallel): each core owns a 32-row band of the 256x256 image,
split into left/right 128-column halves. Host culls + depth-sorts the gaussian
list per half (tiny G=256 arrays), folds opacity into the conic's constant
term, and packs each core's two halves into the 128 SBUF partitions: slots
0-63 carry the left half's gaussians, 64-127 the right half's (63 real + 1
background slot each).

Both halves use the same local pixel coordinate frame, so one [6, 4096]
quadratic pixel basis drives both: column j of every device tile means
"left-half pixel j" for partitions 0-63 and "right-half pixel j" for
partitions 64-127. The compositing matrices are block-diagonal so the two
halves never mix. Device pipeline per 512-column chunk:

  Q  = coef.T @ basis                 (PE; both halves in one fp32 matmul)
  E  = exp(-0.5 Q)                    (ACT; == opa * exp(-0.5 q))
  a  = (E >= thresh) * min(E, 0.99)   (DVE; thresh = exp(-tau'/2), exact mask)
  L  = ln(1 - a)                      (ACT)
  S  = tri.T @ L                      (PE; block-diag exclusive depth cumsum)
  T  = exp(S)                         (ACT; transmittance before each slot)
  W  = T * a                          (DVE)
  img= colors.T @ W                   (PE; block-diag [128,6] -> 2x3 channels)

Background is local slot 63 of each half: Q=0 -> alpha=0.99 exactly, color
bg/0.99, and its S is the full log-transmittance sum, so the colors matmul
emits accum + trans*bg directly.
"""

import numpy as np

H, W_IMG, C = 256, 256, 3
N_CORES = 8
BAND_H = H // N_CORES          # 32 rows per core
HALF_W = W_IMG // 2            # 128 cols per half
HPIX = BAND_H * HALF_W         # 4096 pixels per half
CK = 512                       # pixel chunk (one PSUM bank of fp32)
NCHUNK = HPIX // CK
GH = 64                        # slots per half (63 real + 1 background)
GM = 2 * GH                    # 128 partitions
ALPHA_TH = 1.0 / 255.0
EPS = 1e-8

_PROGRAM_CACHE = {}


def _build_program(dt_q="float32", dt_s="float32", dt_img="bfloat16",
                   with_wlast=False, w_on_gpsimd=False):
    import concourse.bacc as bacc
    import concourse.tile as tile
    import concourse.mybir as mybir

    key = (dt_q, dt_s, dt_img, with_wlast, w_on_gpsimd)
    if key in _PROGRAM_CACHE:
        return _PROGRAM_CACHE[key]

    # Steer the act-table pass to the one set holding BOTH exp and ln, so the
    # per-chunk exp/ln/exp sequence doesn't thrash ~2.7us table reloads: hide
    # Exp/Ln from every other set; the fixpoint then inserts a single load.
    import concourse.bacc as bacc_mod
    from concourse.hw_specs import get_activation_tables as _real_gat

    def _gat_combined(arch):
        out = {}
        for name, funcs in _real_gat(arch).items():
            # Empty every other set so copies/memsets also resolve to the
            # combined set and only one table load is ever emitted.
            out[name] = funcs if name == "natural_log_exp_and_others" else set()
        return out

    bacc_mod.get_activation_tables = _gat_combined

    f32 = mybir.dt.float32
    dq = getattr(mybir.dt, dt_q)
    ds = getattr(mybir.dt, dt_s)
    di = getattr(mybir.dt, dt_img)
    AF = mybir.ActivationFunctionType
    ALU = mybir.AluOpType

    nc = bacc.Bacc("TRN2", target_bir_lowering=False, debug=False)
    basis_d = nc.dram_tensor("basis", [6, HPIX], dq, kind="ExternalInput").ap()
    coef_d = nc.dram_tensor("coef", [6, GM], dq, kind="ExternalInput").ap()
    tri_d = nc.dram_tensor("tri", [GM, GM], ds, kind="ExternalInput").ap()
    # thresh = exp(-0.5*tau'): the mask q' <= tau' becomes E >= thresh, an
    # SBUF-only compare against the already-computed E (exp is monotone).
    thresh_d = nc.dram_tensor("thresh", [GM, 1], f32, kind="ExternalInput").ap()
    colors_d = nc.dram_tensor("colors", [GM, 2 * C], di,
                              kind="ExternalInput").ap()
    img_d = nc.dram_tensor("img", [2 * C, HPIX], f32,
                           kind="ExternalOutput").ap()
    wlast_d = (nc.dram_tensor("wlast", [2, HPIX], di,
                              kind="ExternalOutput").ap()
               if with_wlast else None)

    with tile.TileContext(nc) as tc:
        with (
            tc.tile_pool(name="const", bufs=1) as cpool,
            tc.tile_pool(name="big", bufs=1) as bpool,
            tc.tile_pool(name="work", bufs=4) as wpool,
            tc.tile_pool(name="qps", bufs=4, space="PSUM") as qpool,
            tc.tile_pool(name="sps", bufs=3, space="PSUM") as spool,
            tc.tile_pool(name="ips", bufs=1, space="PSUM") as ipool,
        ):
            ET = mybir.EngineType
            basis_s = cpool.tile_from(basis_d, name="basis_s",
                                      forced_dma_engine=ET.SP)
            coef_s = cpool.tile_from(coef_d, name="coef_s",
                                     forced_dma_engine=ET.Pool)
            tri_s = cpool.tile_from(tri_d, name="tri_s",
                                    forced_dma_engine=ET.Activation)
            thresh_s = cpool.tile_from(thresh_d, name="thresh_s",
                                       forced_dma_engine=ET.Pool)
            colors_s = cpool.tile_from(colors_d, name="colors_s",
                                       forced_dma_engine=ET.Activation)

            w_t = bpool.tile([GM, HPIX], di)

            # PE warm-up: ~3.5us of dummy bf16 matmuls overlapping the input
            # DMAs so the HAM clock gate releases before the first real
            # (fp32, 4 cyc/row) matmul instead of during chunk 0-1.
            z_t = cpool.tile([GM, CK], di)
            nc.vector.memset(z_t[:], 0.0)
            for _ in range(10):
                wm = ipool.tile([GM, CK], f32, tag="img")
                nc.tensor.matmul(wm[:], z_t[:, :GM], z_t[:],
                                 start=True, stop=True)

            # Software-pipelined with skew: PE's in-order stream becomes
            # Q0 Q1 [Q2 S0] [Q3 S1 I0] ... so it never stalls on the
            # ACT/DVE round-trip of the current chunk.
            q_tiles = {}
            s_tiles = {}
            alpha_tiles = {}
            for t in range(NCHUNK + 3):
                if t < NCHUNK:
                    cs = slice(t * CK, (t + 1) * CK)
                    q_ps = qpool.tile([GM, CK], f32, tag="q")
                    nc.tensor.matmul(q_ps[:], coef_s[:], basis_s[:, cs],
                                     start=True, stop=True)
                    q_tiles[t] = q_ps
                if 2 <= t < NCHUNK + 2:
                    i = t - 2
                    q_ps = q_tiles.pop(i)
                    e_t = wpool.tile([GM, CK], f32, tag="e")
                    nc.scalar.activation(e_t[:], q_ps[:], AF.Exp, scale=-0.5)
                    t2 = wpool.tile([GM, CK], f32, tag="t2")
                    nc.vector.tensor_scalar(t2[:], e_t[:], 0.99, None, ALU.min)
                    alpha = wpool.tile([GM, CK], f32, tag="alpha")
                    nc.vector.scalar_tensor_tensor(
                        alpha[:], e_t[:], thresh_s[:], t2[:],
                        ALU.is_ge, ALU.mult)
                    l_t = wpool.tile([GM, CK], ds, tag="l")
                    nc.scalar.activation(l_t[:], alpha[:], AF.Ln,
                                         bias=1.0, scale=-1.0)
                    s_ps = spool.tile([GM, CK], f32, tag="s")
                    nc.tensor.matmul(s_ps[:], tri_s[:], l_t[:],
                                     start=True, stop=True)
                    s_tiles[i] = s_ps
                    alpha_tiles[i] = alpha
                if t >= 3:
                    i = t - 3
                    cs = slice(i * CK, (i + 1) * CK)
                    s_ps = s_tiles.pop(i)
                    alpha = alpha_tiles.pop(i)
                    t_t = wpool.tile([GM, CK], f32, tag="t")
                    nc.scalar.activation(t_t[:], s_ps[:], AF.Exp)
                    use_pool = w_on_gpsimd and i < NCHUNK - 2
                    w_eng = nc.gpsimd if use_pool else nc.vector
                    w_eng.tensor_tensor(w_t[:, cs], t_t[:], alpha[:],
                                        ALU.mult)
                    i_ps = ipool.tile([2 * C, CK], f32, tag="img")
                    nc.tensor.matmul(i_ps[:], colors_s[:], w_t[:, cs],
                                     start=True, stop=True)
                    if i >= NCHUNK - 2:
                        i_sb1 = wpool.tile([2 * C, CK], f32, tag="imgsb1")
                        nc.vector.tensor_copy(i_sb1[:], i_ps[:])
                        nc.sync.dma_start(img_d[:, cs], i_sb1[:])
                    else:
                        if i % 2 == 0:
                            i_sb = wpool.tile([2 * C, 2 * CK], f32,
                                              tag="imgsb")
                        nc.vector.tensor_copy(
                            i_sb[:, (i % 2) * CK:(i % 2 + 1) * CK], i_ps[:])
                        if i % 2 == 1:
                            nc.sync.dma_start(
                                img_d[:, (i - 1) * CK:(i + 1) * CK], i_sb[:])
            if with_wlast:
                nc.sync.dma_start(wlast_d[0:1, :], w_t[GH - 1:GH, :])
                nc.sync.dma_start(wlast_d[1:2, :], w_t[GM - 1:GM, :])

    nc.compile()
    _PROGRAM_CACHE[key] = nc
    return nc


def _host_prep(means2d, conics, colors, opacities, depths, background):
    """Sort by depth, cull per 32x128 half-tile, pack device inputs.

    Returns (in_maps, n_pass): in_maps[p][core] is the input dict for pass p,
    n_pass is 1 unless some half has more than GH-1 surviving gaussians.
    """
    order = np.argsort(depths, kind="stable")
    m = means2d[order].astype(np.float64)
    k = conics[order].astype(np.float64)
    col = colors[order].astype(np.float32)
    o = opacities[order].astype(np.float64)

    a, b, c = k[:, 0], k[:, 1], k[:, 2]
    det = a * c - b * b
    tau = -2.0 * np.log(np.maximum(ALPHA_TH / np.maximum(o, EPS), EPS))
    valid = (o > ALPHA_TH) & (det > EPS) & (a > 0.0) & (c > 0.0) & (tau > 0.0)

    with np.errstate(divide="ignore", invalid="ignore"):
        safe_det = np.where(det > EPS, det, 1.0)
        dy_max = np.sqrt(np.maximum(tau * np.where(valid, a / safe_det, 0.), 0.))
        dx_max = np.sqrt(np.maximum(tau * np.where(valid, c / safe_det, 0.), 0.))
    ln_o = np.log(np.maximum(o, EPS))

    keeps = {}
    for band in range(N_CORES):
        r0 = band * BAND_H
        ky = (valid & (m[:, 1] + dy_max >= r0 + 0.5)
              & (m[:, 1] - dy_max <= r0 + BAND_H - 0.5))
        for xh in range(2):
            c0 = xh * HALF_W
            keeps[(band, xh)] = np.where(
                ky & (m[:, 0] + dx_max >= c0 + 0.5)
                & (m[:, 0] - dx_max <= c0 + HALF_W - 0.5))[0]

    n_pass = max(1, int(np.ceil(
        max(len(kp) for kp in keeps.values()) / (GH - 1))))

    bg32 = background.astype(np.float32) / np.float32(0.99)
    in_maps = []
    for p in range(n_pass):
        last = p == n_pass - 1
        maps = []
        for band in range(N_CORES):
            coef = np.zeros((6, GM), np.float32)
            thresh = np.full((GM, 1), 1e30, np.float32)
            cols = np.zeros((GM, 2 * C), np.float32)
            for xh in range(2):
                keep = keeps[(band, xh)][p * (GH - 1):(p + 1) * (GH - 1)]
                n = len(keep)
                s0 = xh * GH
                ka, kb, kc = a[keep], b[keep], c[keep]
                mx = m[keep, 0] - (xh * HALF_W + HALF_W / 2.0)
                my = m[keep, 1] - band * BAND_H - BAND_H / 2.0
                coef[0, s0:s0 + n] = ka
                coef[1, s0:s0 + n] = 2.0 * kb
                coef[2, s0:s0 + n] = kc
                coef[3, s0:s0 + n] = -2.0 * ka * mx - 2.0 * kb * my
                coef[4, s0:s0 + n] = -2.0 * kb * mx - 2.0 * kc * my
                coef[5, s0:s0 + n] = (ka * mx * mx + 2.0 * kb * mx * my
                                      + kc * my * my - 2.0 * ln_o[keep])
                thresh[s0:s0 + n, 0] = np.exp(
                    -0.5 * (tau[keep] - 2.0 * ln_o[keep])).astype(np.float32)
                cols[s0:s0 + n, xh * C:(xh + 1) * C] = col[keep]
                # background slot: alpha == 0.99, S == full log-transmittance
                thresh[s0 + GH - 1, 0] = 0.0
                coef[:, s0 + GH - 1] = 0.0
                cols[s0 + GH - 1] = 0.0
                if last:
                    cols[s0 + GH - 1, xh * C:(xh + 1) * C] = bg32
            maps.append({"coef": coef, "thresh": thresh, "cols": cols})
        in_maps.append(maps)
    return in_maps, n_pass


def _pixel_basis():
    ys, xs = np.meshgrid(
        np.arange(BAND_H, dtype=np.float32) - (BAND_H / 2.0 - 0.5),
        np.arange(HALF_W, dtype=np.float32) - (HALF_W / 2.0 - 0.5),
        indexing="ij")
    xs = xs.reshape(-1)
    ys = ys.reshape(-1)
    return np.stack([xs * xs, xs * ys, ys * ys, xs, ys,
                     np.ones_like(xs)], 0).astype(np.float32)


def _tri_blockdiag(np_s):
    tri = np.zeros((GM, GM), np.float32)
    blk = np.triu(np.ones((GH, GH), np.float32), 1)
    tri[:GH, :GH] = blk
    tri[GH:, GH:] = blk
    return tri.astype(np_s)


def kernel(means2d, conics, colors, opacities, depths, background,
           dt_q="float32", dt_s="float32", dt_img="bfloat16",
           _trace=False):
    import ml_dtypes
    from concourse.bass_utils import run_bass_kernel_spmd

    maps, n_pass = _host_prep(
        np.asarray(means2d), np.asarray(conics), np.asarray(colors),
        np.asarray(opacities), np.asarray(depths), np.asarray(background))
    nc = _build_program(dt_q, dt_s, dt_img, with_wlast=n_pass > 1)

    np_q = np.float32
    np_s = ml_dtypes.bfloat16 if dt_s == "bfloat16" else np.float32
    np_i = ml_dtypes.bfloat16 if dt_img == "bfloat16" else np.float32
    basis = _pixel_basis().astype(np_q)
    tri = _tri_blockdiag(np_s)

    acc = np.zeros((N_CORES, 2 * C, HPIX), np.float32)
    trans = np.ones((N_CORES, 2, 1, HPIX), np.float32)
    results = None
    for p in range(n_pass):
        in_maps = [{
            "basis": basis,
            "coef": maps[p][core]["coef"].astype(np_q),
            "tri": tri,
            "thresh": maps[p][core]["thresh"],
            "colors": maps[p][core]["cols"].astype(np_i),
        } for core in range(N_CORES)]
        results = run_bass_kernel_spmd(
            nc, in_maps, core_ids=list(range(N_CORES)), trace=_trace)
        for core in range(N_CORES):
            r = results.results[core]
            img = r["img"]
            for xh in range(2):
                acc[core, xh * C:(xh + 1) * C] += (
                    trans[core, xh] * img[xh * C:(xh + 1) * C])
                if n_pass > 1:
                    trans[core, xh] = trans[core, xh] * (
                        r["wlast"][xh:xh + 1].astype(np.float32)
                        / np.float32(0.99))

    out = np.empty((H, W_IMG, C), np.float32)
    for core in range(N_CORES):
        band = acc[core].reshape(2, C, BAND_H, HALF_W)
        r0 = core * BAND_H
        out[r0:r0 + BAND_H, :HALF_W] = band[0].transpose(1, 2, 0)
        out[r0:r0 + BAND_H, HALF_W:] = band[1].transpose(1, 2, 0)
    if _trace:
        return out, results
    return out


# revision 25
# speedup vs baseline: 1.0562x; 1.0562x over previous
"""Memory-efficient Gaussian rasterizer on 8 Trainium2 NeuronCores.

Strategy (tile-parallel): each core owns a 32-row band of the 256x256 image,
split into left/right 128-column halves. Host culls + depth-sorts the gaussian
list per half (tiny G=256 arrays), folds opacity into the conic's constant
term, and packs each core's two halves into the 128 SBUF partitions: slots
0-63 carry the left half's gaussians, 64-127 the right half's (63 real + 1
background slot each).

Both halves use the same local pixel coordinate frame, so one [6, 4096]
quadratic pixel basis drives both: column j of every device tile means
"left-half pixel j" for partitions 0-63 and "right-half pixel j" for
partitions 64-127. The compositing matrices are block-diagonal so the two
halves never mix. Device pipeline per 512-column chunk:

  Q  = coef.T @ basis                 (PE; both halves in one fp32 matmul)
  E  = exp(-0.5 Q)                    (ACT; == opa * exp(-0.5 q))
  a  = (E >= thresh) * min(E, 0.99)   (DVE; thresh = exp(-tau'/2), exact mask)
  L  = ln(1 - a)                      (ACT)
  S  = tri.T @ L                      (PE; block-diag exclusive depth cumsum)
  T  = exp(S)                         (ACT; transmittance before each slot)
  W  = T * a                          (DVE)
  img= colors.T @ W                   (PE; block-diag [128,6] -> 2x3 channels)

Background is local slot 63 of each half: Q=0 -> alpha=0.99 exactly, color
bg/0.99, and its S is the full log-transmittance sum, so the colors matmul
emits accum + trans*bg directly.
"""

import numpy as np

H, W_IMG, C = 256, 256, 3
N_CORES = 8
BAND_H = H // N_CORES          # 32 rows per core
HALF_W = W_IMG // 2            # 128 cols per half
HPIX = BAND_H * HALF_W         # 4096 pixels per half
CK = 512                       # pixel chunk (one PSUM bank of fp32)
NCHUNK = HPIX // CK
GH = 64                        # slots per half (63 real + 1 background)
GM = 2 * GH                    # 128 partitions
ALPHA_TH = 1.0 / 255.0
EPS = 1e-8

_PROGRAM_CACHE = {}


def _build_program(dt_q="float32", dt_s="float32", dt_img="bfloat16",
                   with_wlast=False, w_on_gpsimd=False):
    import concourse.bacc as bacc
    import concourse.tile as tile
    import concourse.mybir as mybir

    key = (dt_q, dt_s, dt_img, with_wlast, w_on_gpsimd)
    if key in _PROGRAM_CACHE:
        return _PROGRAM_CACHE[key]

    # Steer the act-table pass to the one set holding BOTH exp and ln, so the
    # per-chunk exp/ln/exp sequence doesn't thrash ~2.7us table reloads: hide
    # Exp/Ln from every other set; the fixpoint then inserts a single load.
    import concourse.bacc as bacc_mod
    from concourse.hw_specs import get_activation_tables as _real_gat

    def _gat_combined(arch):
        out = {}
        for name, funcs in _real_gat(arch).items():
            # Empty every other set so copies/memsets also resolve to the
            # combined set and only one table load is ever emitted.
            out[name] = funcs if name == "natural_log_exp_and_others" else set()
        return out

    bacc_mod.get_activation_tables = _gat_combined

    f32 = mybir.dt.float32
    dq = getattr(mybir.dt, dt_q)
    ds = getattr(mybir.dt, dt_s)
    di = getattr(mybir.dt, dt_img)
    AF = mybir.ActivationFunctionType
    ALU = mybir.AluOpType

    nc = bacc.Bacc("TRN2", target_bir_lowering=False, debug=False)
    basis_d = nc.dram_tensor("basis", [6, HPIX], dq, kind="ExternalInput").ap()
    coef_d = nc.dram_tensor("coef", [6, GM], dq, kind="ExternalInput").ap()
    tri_d = nc.dram_tensor("tri", [GM, GM], ds, kind="ExternalInput").ap()
    # thresh = exp(-0.5*tau'): the mask q' <= tau' becomes E >= thresh, an
    # SBUF-only compare against the already-computed E (exp is monotone).
    thresh_d = nc.dram_tensor("thresh", [GM, 1], f32, kind="ExternalInput").ap()
    colors_d = nc.dram_tensor("colors", [GM, 2 * C], di,
                              kind="ExternalInput").ap()
    img_d = nc.dram_tensor("img", [2 * C, HPIX], f32,
                           kind="ExternalOutput").ap()
    wlast_d = (nc.dram_tensor("wlast", [2, HPIX], di,
                              kind="ExternalOutput").ap()
               if with_wlast else None)

    with tile.TileContext(nc) as tc:
        with (
            tc.tile_pool(name="const", bufs=1) as cpool,
            tc.tile_pool(name="big", bufs=1) as bpool,
            tc.tile_pool(name="work", bufs=4) as wpool,
            tc.tile_pool(name="qps", bufs=3, space="PSUM") as qpool,
            tc.tile_pool(name="sps", bufs=3, space="PSUM") as spool,
            tc.tile_pool(name="ips", bufs=2, space="PSUM") as ipool,
        ):
            ET = mybir.EngineType
            basis_s = cpool.tile_from(basis_d, name="basis_s",
                                      forced_dma_engine=ET.SP)
            coef_s = cpool.tile_from(coef_d, name="coef_s",
                                     forced_dma_engine=ET.Pool)
            tri_s = cpool.tile_from(tri_d, name="tri_s",
                                    forced_dma_engine=ET.Activation)
            thresh_s = cpool.tile_from(thresh_d, name="thresh_s",
                                       forced_dma_engine=ET.Pool)
            colors_s = cpool.tile_from(colors_d, name="colors_s",
                                       forced_dma_engine=ET.Activation)

            w_t = bpool.tile([GM, HPIX], di)

            # PE warm-up: ~3.5us of dummy bf16 matmuls overlapping the input
            # DMAs so the HAM clock gate releases before the first real
            # (fp32, 4 cyc/row) matmul instead of during chunk 0-1.
            z_t = cpool.tile([GM, CK], di)
            nc.vector.memset(z_t[:], 0.0)
            for _ in range(8):
                wm = ipool.tile([GM, CK], f32, tag="img")
                nc.tensor.matmul(wm[:], z_t[:, :GM], z_t[:],
                                 start=True, stop=True)

            # Software-pipelined with skew: PE's in-order stream becomes
            # Q0 Q1 [Q2 S0] [Q3 S1 I0] ... so it never stalls on the
            # ACT/DVE round-trip of the current chunk.
            q_tiles = {}
            s_tiles = {}
            alpha_tiles = {}
            for t in range(NCHUNK + 3):
                if t < NCHUNK:
                    cs = slice(t * CK, (t + 1) * CK)
                    q_ps = qpool.tile([GM, CK], f32, tag="q")
                    nc.tensor.matmul(q_ps[:], coef_s[:], basis_s[:, cs],
                                     start=True, stop=True)
                    q_tiles[t] = q_ps
                if 2 <= t < NCHUNK + 2:
                    i = t - 2
                    q_ps = q_tiles.pop(i)
                    e_t = wpool.tile([GM, CK], f32, tag="e")
                    nc.scalar.activation(e_t[:], q_ps[:], AF.Exp, scale=-0.5)
                    t2 = wpool.tile([GM, CK], f32, tag="t2")
                    nc.vector.tensor_scalar(t2[:], e_t[:], 0.99, None, ALU.min)
                    alpha = wpool.tile([GM, CK], f32, tag="alpha")
                    nc.vector.scalar_tensor_tensor(
                        alpha[:], e_t[:], thresh_s[:], t2[:],
                        ALU.is_ge, ALU.mult)
                    l_t = wpool.tile([GM, CK], ds, tag="l")
                    nc.scalar.activation(l_t[:], alpha[:], AF.Ln,
                                         bias=1.0, scale=-1.0)
                    s_ps = spool.tile([GM, CK], f32, tag="s")
                    nc.tensor.matmul(s_ps[:], tri_s[:], l_t[:],
                                     start=True, stop=True)
                    s_tiles[i] = s_ps
                    alpha_tiles[i] = alpha
                if t >= 3:
                    i = t - 3
                    cs = slice(i * CK, (i + 1) * CK)
                    s_ps = s_tiles.pop(i)
                    alpha = alpha_tiles.pop(i)
                    t_t = wpool.tile([GM, CK], f32, tag="t")
                    nc.scalar.activation(t_t[:], s_ps[:], AF.Exp)
                    use_pool = w_on_gpsimd and i < NCHUNK - 2
                    w_eng = nc.gpsimd if use_pool else nc.vector
                    w_eng.tensor_tensor(w_t[:, cs], t_t[:], alpha[:],
                                        ALU.mult)
                    i_ps = ipool.tile([2 * C, CK], f32, tag="img")
                    nc.tensor.matmul(i_ps[:], colors_s[:], w_t[:, cs],
                                     start=True, stop=True)
                    if i >= NCHUNK - 2:
                        i_sb1 = wpool.tile([2 * C, CK], f32, tag="imgsb1")
                        nc.vector.tensor_copy(i_sb1[:], i_ps[:])
                        nc.sync.dma_start(img_d[:, cs], i_sb1[:])
                    else:
                        if i % 2 == 0:
                            i_sb = wpool.tile([2 * C, 2 * CK], f32,
                                              tag="imgsb")
                        nc.vector.tensor_copy(
                            i_sb[:, (i % 2) * CK:(i % 2 + 1) * CK], i_ps[:])
                        if i % 2 == 1:
                            nc.sync.dma_start(
                                img_d[:, (i - 1) * CK:(i + 1) * CK], i_sb[:])
            if with_wlast:
                nc.sync.dma_start(wlast_d[0:1, :], w_t[GH - 1:GH, :])
                nc.sync.dma_start(wlast_d[1:2, :], w_t[GM - 1:GM, :])

    nc.compile()
    _PROGRAM_CACHE[key] = nc
    return nc


def _host_prep(means2d, conics, colors, opacities, depths, background):
    """Sort by depth, cull per 32x128 half-tile, pack device inputs.

    Returns (in_maps, n_pass): in_maps[p][core] is the input dict for pass p,
    n_pass is 1 unless some half has more than GH-1 surviving gaussians.
    """
    order = np.argsort(depths, kind="stable")
    m = means2d[order].astype(np.float64)
    k = conics[order].astype(np.float64)
    col = colors[order].astype(np.float32)
    o = opacities[order].astype(np.float64)

    a, b, c = k[:, 0], k[:, 1], k[:, 2]
    det = a * c - b * b
    tau = -2.0 * np.log(np.maximum(ALPHA_TH / np.maximum(o, EPS), EPS))
    valid = (o > ALPHA_TH) & (det > EPS) & (a > 0.0) & (c > 0.0) & (tau > 0.0)

    with np.errstate(divide="ignore", invalid="ignore"):
        safe_det = np.where(det > EPS, det, 1.0)
        dy_max = np.sqrt(np.maximum(tau * np.where(valid, a / safe_det, 0.), 0.))
        dx_max = np.sqrt(np.maximum(tau * np.where(valid, c / safe_det, 0.), 0.))
    ln_o = np.log(np.maximum(o, EPS))

    keeps = {}
    for band in range(N_CORES):
        r0 = band * BAND_H
        ky = (valid & (m[:, 1] + dy_max >= r0 + 0.5)
              & (m[:, 1] - dy_max <= r0 + BAND_H - 0.5))
        for xh in range(2):
            c0 = xh * HALF_W
            keeps[(band, xh)] = np.where(
                ky & (m[:, 0] + dx_max >= c0 + 0.5)
                & (m[:, 0] - dx_max <= c0 + HALF_W - 0.5))[0]

    n_pass = max(1, int(np.ceil(
        max(len(kp) for kp in keeps.values()) / (GH - 1))))

    bg32 = background.astype(np.float32) / np.float32(0.99)
    in_maps = []
    for p in range(n_pass):
        last = p == n_pass - 1
        maps = []
        for band in range(N_CORES):
            coef = np.zeros((6, GM), np.float32)
            thresh = np.full((GM, 1), 1e30, np.float32)
            cols = np.zeros((GM, 2 * C), np.float32)
            for xh in range(2):
                keep = keeps[(band, xh)][p * (GH - 1):(p + 1) * (GH - 1)]
                n = len(keep)
                s0 = xh * GH
                ka, kb, kc = a[keep], b[keep], c[keep]
                mx = m[keep, 0] - (xh * HALF_W + HALF_W / 2.0)
                my = m[keep, 1] - band * BAND_H - BAND_H / 2.0
                coef[0, s0:s0 + n] = ka
                coef[1, s0:s0 + n] = 2.0 * kb
                coef[2, s0:s0 + n] = kc
                coef[3, s0:s0 + n] = -2.0 * ka * mx - 2.0 * kb * my
                coef[4, s0:s0 + n] = -2.0 * kb * mx - 2.0 * kc * my
                coef[5, s0:s0 + n] = (ka * mx * mx + 2.0 * kb * mx * my
                                      + kc * my * my - 2.0 * ln_o[keep])
                thresh[s0:s0 + n, 0] = np.exp(
                    -0.5 * (tau[keep] - 2.0 * ln_o[keep])).astype(np.float32)
                cols[s0:s0 + n, xh * C:(xh + 1) * C] = col[keep]
                # background slot: alpha == 0.99, S == full log-transmittance
                thresh[s0 + GH - 1, 0] = 0.0
                coef[:, s0 + GH - 1] = 0.0
                cols[s0 + GH - 1] = 0.0
                if last:
                    cols[s0 + GH - 1, xh * C:(xh + 1) * C] = bg32
            maps.append({"coef": coef, "thresh": thresh, "cols": cols})
        in_maps.append(maps)
    return in_maps, n_pass


def _pixel_basis():
    ys, xs = np.meshgrid(
        np.arange(BAND_H, dtype=np.float32) - (BAND_H / 2.0 - 0.5),
        np.arange(HALF_W, dtype=np.float32) - (HALF_W / 2.0 - 0.5),
        indexing="ij")
    xs = xs.reshape(-1)
    ys = ys.reshape(-1)
    return np.stack([xs * xs, xs * ys, ys * ys, xs, ys,
                     np.ones_like(xs)], 0).astype(np.float32)


def _tri_blockdiag(np_s):
    tri = np.zeros((GM, GM), np.float32)
    blk = np.triu(np.ones((GH, GH), np.float32), 1)
    tri[:GH, :GH] = blk
    tri[GH:, GH:] = blk
    return tri.astype(np_s)


def kernel(means2d, conics, colors, opacities, depths, background,
           dt_q="float32", dt_s="float32", dt_img="bfloat16",
           _trace=False):
    import ml_dtypes
    from concourse.bass_utils import run_bass_kernel_spmd

    maps, n_pass = _host_prep(
        np.asarray(means2d), np.asarray(conics), np.asarray(colors),
        np.asarray(opacities), np.asarray(depths), np.asarray(background))
    nc = _build_program(dt_q, dt_s, dt_img, with_wlast=n_pass > 1)

    np_q = np.float32
    np_s = ml_dtypes.bfloat16 if dt_s == "bfloat16" else np.float32
    np_i = ml_dtypes.bfloat16 if dt_img == "bfloat16" else np.float32
    basis = _pixel_basis().astype(np_q)
    tri = _tri_blockdiag(np_s)

    acc = np.zeros((N_CORES, 2 * C, HPIX), np.float32)
    trans = np.ones((N_CORES, 2, 1, HPIX), np.float32)
    results = None
    for p in range(n_pass):
        in_maps = [{
            "basis": basis,
            "coef": maps[p][core]["coef"].astype(np_q),
            "tri": tri,
            "thresh": maps[p][core]["thresh"],
            "colors": maps[p][core]["cols"].astype(np_i),
        } for core in range(N_CORES)]
        results = run_bass_kernel_spmd(
            nc, in_maps, core_ids=list(range(N_CORES)), trace=_trace)
        for core in range(N_CORES):
            r = results.results[core]
            img = r["img"]
            for xh in range(2):
                acc[core, xh * C:(xh + 1) * C] += (
                    trans[core, xh] * img[xh * C:(xh + 1) * C])
                if n_pass > 1:
                    trans[core, xh] = trans[core, xh] * (
                        r["wlast"][xh:xh + 1].astype(np.float32)
                        / np.float32(0.99))

    out = np.empty((H, W_IMG, C), np.float32)
    for core in range(N_CORES):
        band = acc[core].reshape(2, C, BAND_H, HALF_W)
        r0 = core * BAND_H
        out[r0:r0 + BAND_H, :HALF_W] = band[0].transpose(1, 2, 0)
        out[r0:r0 + BAND_H, HALF_W:] = band[1].transpose(1, 2, 0)
    if _trace:
        return out, results
    return out


# revision 38
# speedup vs baseline: 1.1224x; 1.0626x over previous
"""Memory-efficient Gaussian rasterizer on 8 Trainium2 NeuronCores.

Strategy (tile-parallel): each core owns a 32-row band of the 256x256 image,
split into left/right 128-column halves. Host culls + depth-sorts the gaussian
list per half (tiny G=256 arrays), folds opacity into the conic's constant
term, and packs each core's two halves into the 128 SBUF partitions: slots
0-63 carry the left half's gaussians, 64-127 the right half's (63 real + 1
background slot each).

Both halves use the same local pixel coordinate frame, so one [6, 4096]
quadratic pixel basis drives both: column j of every device tile means
"left-half pixel j" for partitions 0-63 and "right-half pixel j" for
partitions 64-127. The compositing matrices are block-diagonal so the two
halves never mix. Device pipeline per 512-column chunk:

  Q  = coef.T @ basis                 (PE; both halves in one fp32 matmul)
  E  = exp(-0.5 Q)                    (ACT; == opa * exp(-0.5 q))
  a  = (E >= thresh) * min(E, 0.99)   (DVE; thresh = exp(-tau'/2), exact mask)
  L  = ln(1 - a)                      (ACT)
  S  = tri.T @ L                      (PE; block-diag exclusive depth cumsum)
  T  = exp(S)                         (ACT; transmittance before each slot)
  W  = T * a                          (DVE)
  img= colors.T @ W                   (PE; block-diag [128,6] -> 2x3 channels)

Background is local slot 63 of each half: Q=0 -> alpha=0.99 exactly, color
bg/0.99, and its S is the full log-transmittance sum, so the colors matmul
emits accum + trans*bg directly.
"""

import numpy as np

H, W_IMG, C = 256, 256, 3
N_CORES = 8
BAND_H = H // N_CORES          # 32 rows per core
HALF_W = W_IMG // 2            # 128 cols per half
HPIX = BAND_H * HALF_W         # 4096 pixels per half
CK = 512                       # pixel chunk (one PSUM bank of fp32)
NCHUNK = HPIX // CK
GH = 64                        # slots per half (63 real + 1 background)
GM = 2 * GH                    # 128 partitions
ALPHA_TH = 1.0 / 255.0
EPS = 1e-8

_PROGRAM_CACHE = {}


def _build_program(dt_q="float32", dt_s="float32", dt_img="bfloat16",
                   with_wlast=False, w_on_gpsimd=False):
    import concourse.bacc as bacc
    import concourse.tile as tile
    import concourse.mybir as mybir

    key = (dt_q, dt_s, dt_img, with_wlast, w_on_gpsimd)
    if key in _PROGRAM_CACHE:
        return _PROGRAM_CACHE[key]

    # Steer the act-table pass to the one set holding BOTH exp and ln, so the
    # per-chunk exp/ln/exp sequence doesn't thrash ~2.7us table reloads: hide
    # Exp/Ln from every other set; the fixpoint then inserts a single load.
    import concourse.bacc as bacc_mod
    from concourse.hw_specs import get_activation_tables as _real_gat

    def _gat_combined(arch):
        out = {}
        for name, funcs in _real_gat(arch).items():
            # Empty every other set so copies/memsets also resolve to the
            # combined set and only one table load is ever emitted.
            out[name] = funcs if name == "natural_log_exp_and_others" else set()
        return out

    bacc_mod.get_activation_tables = _gat_combined

    f32 = mybir.dt.float32
    dq = getattr(mybir.dt, dt_q)
    ds = getattr(mybir.dt, dt_s)
    di = getattr(mybir.dt, dt_img)
    AF = mybir.ActivationFunctionType
    ALU = mybir.AluOpType

    nc = bacc.Bacc("TRN2", target_bir_lowering=False, debug=False)
    basis_d = nc.dram_tensor("basis", [6, HPIX], dq, kind="ExternalInput").ap()
    coef_d = nc.dram_tensor("coef", [6, GM], dq, kind="ExternalInput").ap()
    tri_d = nc.dram_tensor("tri", [GM, GM], ds, kind="ExternalInput").ap()
    # thresh = exp(-0.5*tau'): the mask q' <= tau' becomes E >= thresh, an
    # SBUF-only compare against the already-computed E (exp is monotone).
    thresh_d = nc.dram_tensor("thresh", [GM, 1], f32, kind="ExternalInput").ap()
    colors_d = nc.dram_tensor("colors", [GM, 2 * C], di,
                              kind="ExternalInput").ap()
    img_d = nc.dram_tensor("img", [2 * C, HPIX], f32,
                           kind="ExternalOutput").ap()
    wlast_d = (nc.dram_tensor("wlast", [2, HPIX], di,
                              kind="ExternalOutput").ap()
               if with_wlast else None)

    with tile.TileContext(nc) as tc:
        with (
            tc.tile_pool(name="const", bufs=1) as cpool,
            tc.tile_pool(name="big", bufs=1) as bpool,
            tc.tile_pool(name="work", bufs=5) as wpool,
            tc.tile_pool(name="qps", bufs=3, space="PSUM") as qpool,
            tc.tile_pool(name="sps", bufs=3, space="PSUM") as spool,
            tc.tile_pool(name="ips", bufs=2, space="PSUM") as ipool,
        ):
            ET = mybir.EngineType
            z_t = cpool.tile([GM, CK], di)
            nc.gpsimd.memset(z_t[:], 0.0)
            basis_s = cpool.tile_from(basis_d, name="basis_s",
                                      forced_dma_engine=ET.SP)
            coef_s = cpool.tile_from(coef_d, name="coef_s",
                                     forced_dma_engine=ET.SP)
            tri_s = cpool.tile_from(tri_d, name="tri_s",
                                    forced_dma_engine=ET.Activation)
            thresh_s = cpool.tile_from(thresh_d, name="thresh_s",
                                       forced_dma_engine=ET.Pool)
            colors_s = cpool.tile_from(colors_d, name="colors_s",
                                       forced_dma_engine=ET.Activation)

            w_t = bpool.tile([GM, HPIX], di)

            # PE warm-up: dummy bf16 matmuls bridging the input-DMA window so
            # PE is continuously busy from ~0.6us; the first cold real matmuls
            # then finish filling the HAM activity window and the clock gate
            # releases mid-stream.
            for _ in range(4):
                wm = ipool.tile([GM, CK // 2], f32, tag="img")
                nc.tensor.matmul(wm[:], z_t[:, :GM], z_t[:, :CK // 2],
                                 start=True, stop=True)

            # Software-pipelined with skew: PE's in-order stream becomes
            # Q0 Q1 [Q2 S0] [Q3 S1 I0] ... so it never stalls on the
            # ACT/DVE round-trip of the current chunk.
            q_tiles = {}
            s_tiles = {}
            alpha_tiles = {}
            for t in range(NCHUNK + 3):
                if t < NCHUNK:
                    cs = slice(t * CK, (t + 1) * CK)
                    q_ps = qpool.tile([GM, CK], f32, tag="q")
                    nc.tensor.matmul(q_ps[:], coef_s[:], basis_s[:, cs],
                                     start=True, stop=True)
                    q_tiles[t] = q_ps
                if 2 <= t < NCHUNK + 2:
                    i = t - 2
                    q_ps = q_tiles.pop(i)
                    e_t = wpool.tile([GM, CK], f32, tag="e")
                    nc.scalar.activation(e_t[:], q_ps[:], AF.Exp, scale=-0.5)
                    t2 = wpool.tile([GM, CK], f32, tag="t2")
                    nc.vector.tensor_scalar(t2[:], e_t[:], 0.99, None, ALU.min)
                    alpha = wpool.tile([GM, CK], f32, tag="alpha")
                    nc.vector.scalar_tensor_tensor(
                        alpha[:], e_t[:], thresh_s[:], t2[:],
                        ALU.is_ge, ALU.mult)
                    l_t = wpool.tile([GM, CK], ds, tag="l")
                    nc.scalar.activation(l_t[:], alpha[:], AF.Ln,
                                         bias=1.0, scale=-1.0)
                    s_ps = spool.tile([GM, CK], f32, tag="s")
                    nc.tensor.matmul(s_ps[:], tri_s[:], l_t[:],
                                     start=True, stop=True)
                    s_tiles[i] = s_ps
                    alpha_tiles[i] = alpha
                if t >= 3:
                    i = t - 3
                    cs = slice(i * CK, (i + 1) * CK)
                    s_ps = s_tiles.pop(i)
                    alpha = alpha_tiles.pop(i)
                    t_t = wpool.tile([GM, CK], f32, tag="t")
                    nc.scalar.activation(t_t[:], s_ps[:], AF.Exp)
                    use_pool = w_on_gpsimd and i < NCHUNK - 2
                    w_eng = nc.gpsimd if use_pool else nc.vector
                    w_eng.tensor_tensor(w_t[:, cs], t_t[:], alpha[:],
                                        ALU.mult)
                    i_ps = ipool.tile([2 * C, CK], f32, tag="img")
                    nc.tensor.matmul(i_ps[:], colors_s[:], w_t[:, cs],
                                     start=True, stop=True)
                    i_sb = wpool.tile([2 * C, CK], f32, tag="imgsb")
                    if i in (3,):
                        nc.scalar.copy(i_sb[:], i_ps[:])
                    else:
                        nc.vector.tensor_copy(i_sb[:], i_ps[:])
                    nc.sync.dma_start(img_d[:, cs], i_sb[:])
            if with_wlast:
                nc.sync.dma_start(wlast_d[0:1, :], w_t[GH - 1:GH, :])
                nc.sync.dma_start(wlast_d[1:2, :], w_t[GM - 1:GM, :])

    nc.compile()
    _PROGRAM_CACHE[key] = nc
    return nc


def _host_prep(means2d, conics, colors, opacities, depths, background):
    """Sort by depth, cull per 32x128 half-tile, pack device inputs.

    Returns (in_maps, n_pass): in_maps[p][core] is the input dict for pass p,
    n_pass is 1 unless some half has more than GH-1 surviving gaussians.
    """
    order = np.argsort(depths, kind="stable")
    m = means2d[order].astype(np.float64)
    k = conics[order].astype(np.float64)
    col = colors[order].astype(np.float32)
    o = opacities[order].astype(np.float64)

    a, b, c = k[:, 0], k[:, 1], k[:, 2]
    det = a * c - b * b
    tau = -2.0 * np.log(np.maximum(ALPHA_TH / np.maximum(o, EPS), EPS))
    valid = (o > ALPHA_TH) & (det > EPS) & (a > 0.0) & (c > 0.0) & (tau > 0.0)

    with np.errstate(divide="ignore", invalid="ignore"):
        safe_det = np.where(det > EPS, det, 1.0)
        dy_max = np.sqrt(np.maximum(tau * np.where(valid, a / safe_det, 0.), 0.))
        dx_max = np.sqrt(np.maximum(tau * np.where(valid, c / safe_det, 0.), 0.))
    ln_o = np.log(np.maximum(o, EPS))

    keeps = {}
    for band in range(N_CORES):
        r0 = band * BAND_H
        ky = (valid & (m[:, 1] + dy_max >= r0 + 0.5)
              & (m[:, 1] - dy_max <= r0 + BAND_H - 0.5))
        for xh in range(2):
            c0 = xh * HALF_W
            keeps[(band, xh)] = np.where(
                ky & (m[:, 0] + dx_max >= c0 + 0.5)
                & (m[:, 0] - dx_max <= c0 + HALF_W - 0.5))[0]

    n_pass = max(1, int(np.ceil(
        max(len(kp) for kp in keeps.values()) / (GH - 1))))

    bg32 = background.astype(np.float32) / np.float32(0.99)
    in_maps = []
    for p in range(n_pass):
        last = p == n_pass - 1
        maps = []
        for band in range(N_CORES):
            coef = np.zeros((6, GM), np.float32)
            thresh = np.full((GM, 1), 1e30, np.float32)
            cols = np.zeros((GM, 2 * C), np.float32)
            for xh in range(2):
                keep = keeps[(band, xh)][p * (GH - 1):(p + 1) * (GH - 1)]
                n = len(keep)
                s0 = xh * GH
                ka, kb, kc = a[keep], b[keep], c[keep]
                mx = m[keep, 0] - (xh * HALF_W + HALF_W / 2.0)
                my = m[keep, 1] - band * BAND_H - BAND_H / 2.0
                coef[0, s0:s0 + n] = ka
                coef[1, s0:s0 + n] = 2.0 * kb
                coef[2, s0:s0 + n] = kc
                coef[3, s0:s0 + n] = -2.0 * ka * mx - 2.0 * kb * my
                coef[4, s0:s0 + n] = -2.0 * kb * mx - 2.0 * kc * my
                coef[5, s0:s0 + n] = (ka * mx * mx + 2.0 * kb * mx * my
                                      + kc * my * my - 2.0 * ln_o[keep])
                thresh[s0:s0 + n, 0] = np.exp(
                    -0.5 * (tau[keep] - 2.0 * ln_o[keep])).astype(np.float32)
                cols[s0:s0 + n, xh * C:(xh + 1) * C] = col[keep]
                # background slot: alpha == 0.99, S == full log-transmittance
                thresh[s0 + GH - 1, 0] = 0.0
                coef[:, s0 + GH - 1] = 0.0
                cols[s0 + GH - 1] = 0.0
                if last:
                    cols[s0 + GH - 1, xh * C:(xh + 1) * C] = bg32
            maps.append({"coef": coef, "thresh": thresh, "cols": cols})
        in_maps.append(maps)
    return in_maps, n_pass


def _pixel_basis():
    ys, xs = np.meshgrid(
        np.arange(BAND_H, dtype=np.float32) - (BAND_H / 2.0 - 0.5),
        np.arange(HALF_W, dtype=np.float32) - (HALF_W / 2.0 - 0.5),
        indexing="ij")
    xs = xs.reshape(-1)
    ys = ys.reshape(-1)
    return np.stack([xs * xs, xs * ys, ys * ys, xs, ys,
                     np.ones_like(xs)], 0).astype(np.float32)


def _tri_blockdiag(np_s):
    tri = np.zeros((GM, GM), np.float32)
    blk = np.triu(np.ones((GH, GH), np.float32), 1)
    tri[:GH, :GH] = blk
    tri[GH:, GH:] = blk
    return tri.astype(np_s)


def kernel(means2d, conics, colors, opacities, depths, background,
           dt_q="float32", dt_s="float32", dt_img="bfloat16",
           _trace=False):
    import ml_dtypes
    from concourse.bass_utils import run_bass_kernel_spmd

    maps, n_pass = _host_prep(
        np.asarray(means2d), np.asarray(conics), np.asarray(colors),
        np.asarray(opacities), np.asarray(depths), np.asarray(background))
    nc = _build_program(dt_q, dt_s, dt_img, with_wlast=n_pass > 1)

    np_q = np.float32
    np_s = ml_dtypes.bfloat16 if dt_s == "bfloat16" else np.float32
    np_i = ml_dtypes.bfloat16 if dt_img == "bfloat16" else np.float32
    basis = _pixel_basis().astype(np_q)
    tri = _tri_blockdiag(np_s)

    acc = np.zeros((N_CORES, 2 * C, HPIX), np.float32)
    trans = np.ones((N_CORES, 2, 1, HPIX), np.float32)
    results = None
    for p in range(n_pass):
        in_maps = [{
            "basis": basis,
            "coef": maps[p][core]["coef"].astype(np_q),
            "tri": tri,
            "thresh": maps[p][core]["thresh"],
            "colors": maps[p][core]["cols"].astype(np_i),
        } for core in range(N_CORES)]
        results = run_bass_kernel_spmd(
            nc, in_maps, core_ids=list(range(N_CORES)), trace=_trace)
        for core in range(N_CORES):
            r = results.results[core]
            img = r["img"]
            for xh in range(2):
                acc[core, xh * C:(xh + 1) * C] += (
                    trans[core, xh] * img[xh * C:(xh + 1) * C])
                if n_pass > 1:
                    trans[core, xh] = trans[core, xh] * (
                        r["wlast"][xh:xh + 1].astype(np.float32)
                        / np.float32(0.99))

    out = np.empty((H, W_IMG, C), np.float32)
    for core in range(N_CORES):
        band = acc[core].reshape(2, C, BAND_H, HALF_W)
        r0 = core * BAND_H
        out[r0:r0 + BAND_H, :HALF_W] = band[0].transpose(1, 2, 0)
        out[r0:r0 + BAND_H, HALF_W:] = band[1].transpose(1, 2, 0)
    if _trace:
        return out, results
    return out


# revision 39
# speedup vs baseline: 1.1377x; 1.0137x over previous
"""Memory-efficient Gaussian rasterizer on 8 Trainium2 NeuronCores.

Strategy (tile-parallel): each core owns a 32-row band of the 256x256 image,
split into left/right 128-column halves. Host culls + depth-sorts the gaussian
list per half (tiny G=256 arrays), folds opacity into the conic's constant
term, and packs each core's two halves into the 128 SBUF partitions: slots
0-63 carry the left half's gaussians, 64-127 the right half's (63 real + 1
background slot each).

Both halves use the same local pixel coordinate frame, so one [6, 4096]
quadratic pixel basis drives both: column j of every device tile means
"left-half pixel j" for partitions 0-63 and "right-half pixel j" for
partitions 64-127. The compositing matrices are block-diagonal so the two
halves never mix. Device pipeline per 512-column chunk:

  Q  = coef.T @ basis                 (PE; both halves in one fp32 matmul)
  E  = exp(-0.5 Q)                    (ACT; == opa * exp(-0.5 q))
  a  = (E >= thresh) * min(E, 0.99)   (DVE; thresh = exp(-tau'/2), exact mask)
  L  = ln(1 - a)                      (ACT)
  S  = tri.T @ L                      (PE; block-diag exclusive depth cumsum)
  T  = exp(S)                         (ACT; transmittance before each slot)
  W  = T * a                          (DVE)
  img= colors.T @ W                   (PE; block-diag [128,6] -> 2x3 channels)

Background is local slot 63 of each half: Q=0 -> alpha=0.99 exactly, color
bg/0.99, and its S is the full log-transmittance sum, so the colors matmul
emits accum + trans*bg directly.
"""

import numpy as np

H, W_IMG, C = 256, 256, 3
N_CORES = 8
BAND_H = H // N_CORES          # 32 rows per core
HALF_W = W_IMG // 2            # 128 cols per half
HPIX = BAND_H * HALF_W         # 4096 pixels per half
CK = 512                       # pixel chunk (one PSUM bank of fp32)
NCHUNK = HPIX // CK
GH = 64                        # slots per half (63 real + 1 background)
GM = 2 * GH                    # 128 partitions
ALPHA_TH = 1.0 / 255.0
EPS = 1e-8

_PROGRAM_CACHE = {}


def _build_program(dt_q="float32", dt_s="float32", dt_img="bfloat16",
                   with_wlast=False, w_on_gpsimd=False):
    import concourse.bacc as bacc
    import concourse.tile as tile
    import concourse.mybir as mybir

    key = (dt_q, dt_s, dt_img, with_wlast, w_on_gpsimd)
    if key in _PROGRAM_CACHE:
        return _PROGRAM_CACHE[key]

    # Steer the act-table pass to the one set holding BOTH exp and ln, so the
    # per-chunk exp/ln/exp sequence doesn't thrash ~2.7us table reloads: hide
    # Exp/Ln from every other set; the fixpoint then inserts a single load.
    import concourse.bacc as bacc_mod
    from concourse.hw_specs import get_activation_tables as _real_gat

    def _gat_combined(arch):
        out = {}
        for name, funcs in _real_gat(arch).items():
            # Empty every other set so copies/memsets also resolve to the
            # combined set and only one table load is ever emitted.
            out[name] = funcs if name == "natural_log_exp_and_others" else set()
        return out

    bacc_mod.get_activation_tables = _gat_combined

    f32 = mybir.dt.float32
    dq = getattr(mybir.dt, dt_q)
    ds = getattr(mybir.dt, dt_s)
    di = getattr(mybir.dt, dt_img)
    AF = mybir.ActivationFunctionType
    ALU = mybir.AluOpType

    nc = bacc.Bacc("TRN2", target_bir_lowering=False, debug=False)
    basis_d = nc.dram_tensor("basis", [6, HPIX], dq, kind="ExternalInput").ap()
    coef_d = nc.dram_tensor("coef", [6, GM], dq, kind="ExternalInput").ap()
    tri_d = nc.dram_tensor("tri", [GM, GM], ds, kind="ExternalInput").ap()
    # thresh = exp(-0.5*tau'): the mask q' <= tau' becomes E >= thresh, an
    # SBUF-only compare against the already-computed E (exp is monotone).
    thresh_d = nc.dram_tensor("thresh", [GM, 1], f32, kind="ExternalInput").ap()
    colors_d = nc.dram_tensor("colors", [GM, 2 * C], di,
                              kind="ExternalInput").ap()
    img_d = nc.dram_tensor("img", [2 * C, HPIX], f32,
                           kind="ExternalOutput").ap()
    wlast_d = (nc.dram_tensor("wlast", [2, HPIX], di,
                              kind="ExternalOutput").ap()
               if with_wlast else None)

    with tile.TileContext(nc) as tc:
        with (
            tc.tile_pool(name="const", bufs=1) as cpool,
            tc.tile_pool(name="big", bufs=1) as bpool,
            tc.tile_pool(name="work", bufs=5) as wpool,
            tc.tile_pool(name="qps", bufs=3, space="PSUM") as qpool,
            tc.tile_pool(name="sps", bufs=3, space="PSUM") as spool,
            tc.tile_pool(name="ips", bufs=2, space="PSUM") as ipool,
        ):
            ET = mybir.EngineType
            z_t = cpool.tile([GM, CK], di)
            nc.gpsimd.memset(z_t[:], 0.0)
            basis_s = cpool.tile_from(basis_d, name="basis_s",
                                      forced_dma_engine=ET.SP)
            coef_s = cpool.tile_from(coef_d, name="coef_s",
                                     forced_dma_engine=ET.SP)
            tri_s = cpool.tile_from(tri_d, name="tri_s",
                                    forced_dma_engine=ET.Activation)
            thresh_s = cpool.tile_from(thresh_d, name="thresh_s",
                                       forced_dma_engine=ET.Pool)
            colors_s = cpool.tile_from(colors_d, name="colors_s",
                                       forced_dma_engine=ET.Activation)

            w_t = bpool.tile([GM, HPIX], di)

            # PE warm-up: dummy bf16 matmuls bridging the input-DMA window so
            # PE is continuously busy from ~0.6us; the first cold real matmuls
            # then finish filling the HAM activity window and the clock gate
            # releases mid-stream.
            for _ in range(4):
                wm = ipool.tile([GM, CK // 2], f32, tag="img")
                nc.tensor.matmul(wm[:], z_t[:, :GM], z_t[:, :CK // 2],
                                 start=True, stop=True)

            # Software-pipelined with skew: PE's in-order stream becomes
            # Q0 Q1 [Q2 S0] [Q3 S1 I0] ... so it never stalls on the
            # ACT/DVE round-trip of the current chunk.
            chunks = ([(k * CK, CK) for k in range(NCHUNK - 1)]
                      + [((NCHUNK - 1) * CK, CK // 2),
                         ((NCHUNK - 1) * CK + CK // 2, CK // 2)])
            NC2 = len(chunks)
            q_tiles = {}
            s_tiles = {}
            alpha_tiles = {}
            for t in range(NC2 + 3):
                if t < NC2:
                    off, sz = chunks[t]
                    cs = slice(off, off + sz)
                    q_ps = qpool.tile([GM, sz], f32, tag="q")
                    nc.tensor.matmul(q_ps[:], coef_s[:], basis_s[:, cs],
                                     start=True, stop=True)
                    q_tiles[t] = q_ps
                if 2 <= t < NC2 + 2:
                    i = t - 2
                    _, sz = chunks[i]
                    q_ps = q_tiles.pop(i)
                    e_t = wpool.tile([GM, sz], f32, tag="e")
                    nc.scalar.activation(e_t[:], q_ps[:], AF.Exp, scale=-0.5)
                    t2 = wpool.tile([GM, sz], f32, tag="t2")
                    nc.vector.tensor_scalar(t2[:], e_t[:], 0.99, None, ALU.min)
                    alpha = wpool.tile([GM, sz], f32, tag="alpha")
                    nc.vector.scalar_tensor_tensor(
                        alpha[:], e_t[:], thresh_s[:], t2[:],
                        ALU.is_ge, ALU.mult)
                    l_t = wpool.tile([GM, sz], ds, tag="l")
                    nc.scalar.activation(l_t[:], alpha[:], AF.Ln,
                                         bias=1.0, scale=-1.0)
                    s_ps = spool.tile([GM, sz], f32, tag="s")
                    nc.tensor.matmul(s_ps[:], tri_s[:], l_t[:],
                                     start=True, stop=True)
                    s_tiles[i] = s_ps
                    alpha_tiles[i] = alpha
                if t >= 3:
                    i = t - 3
                    off, sz = chunks[i]
                    cs = slice(off, off + sz)
                    s_ps = s_tiles.pop(i)
                    alpha = alpha_tiles.pop(i)
                    t_t = wpool.tile([GM, sz], f32, tag="t")
                    nc.scalar.activation(t_t[:], s_ps[:], AF.Exp)
                    use_pool = w_on_gpsimd and i < NC2 - 2
                    w_eng = nc.gpsimd if use_pool else nc.vector
                    w_eng.tensor_tensor(w_t[:, cs], t_t[:], alpha[:],
                                        ALU.mult)
                    i_ps = ipool.tile([2 * C, sz], f32, tag="img")
                    nc.tensor.matmul(i_ps[:], colors_s[:], w_t[:, cs],
                                     start=True, stop=True)
                    i_sb = wpool.tile([2 * C, sz], f32, tag="imgsb")
                    if i in (3,):
                        nc.scalar.copy(i_sb[:], i_ps[:])
                    else:
                        nc.vector.tensor_copy(i_sb[:], i_ps[:])
                    nc.sync.dma_start(img_d[:, cs], i_sb[:])
            if with_wlast:
                nc.sync.dma_start(wlast_d[0:1, :], w_t[GH - 1:GH, :])
                nc.sync.dma_start(wlast_d[1:2, :], w_t[GM - 1:GM, :])

    nc.compile()
    _PROGRAM_CACHE[key] = nc
    return nc


def _host_prep(means2d, conics, colors, opacities, depths, background):
    """Sort by depth, cull per 32x128 half-tile, pack device inputs.

    Returns (in_maps, n_pass): in_maps[p][core] is the input dict for pass p,
    n_pass is 1 unless some half has more than GH-1 surviving gaussians.
    """
    order = np.argsort(depths, kind="stable")
    m = means2d[order].astype(np.float64)
    k = conics[order].astype(np.float64)
    col = colors[order].astype(np.float32)
    o = opacities[order].astype(np.float64)

    a, b, c = k[:, 0], k[:, 1], k[:, 2]
    det = a * c - b * b
    tau = -2.0 * np.log(np.maximum(ALPHA_TH / np.maximum(o, EPS), EPS))
    valid = (o > ALPHA_TH) & (det > EPS) & (a > 0.0) & (c > 0.0) & (tau > 0.0)

    with np.errstate(divide="ignore", invalid="ignore"):
        safe_det = np.where(det > EPS, det, 1.0)
        dy_max = np.sqrt(np.maximum(tau * np.where(valid, a / safe_det, 0.), 0.))
        dx_max = np.sqrt(np.maximum(tau * np.where(valid, c / safe_det, 0.), 0.))
    ln_o = np.log(np.maximum(o, EPS))

    keeps = {}
    for band in range(N_CORES):
        r0 = band * BAND_H
        ky = (valid & (m[:, 1] + dy_max >= r0 + 0.5)
              & (m[:, 1] - dy_max <= r0 + BAND_H - 0.5))
        for xh in range(2):
            c0 = xh * HALF_W
            keeps[(band, xh)] = np.where(
                ky & (m[:, 0] + dx_max >= c0 + 0.5)
                & (m[:, 0] - dx_max <= c0 + HALF_W - 0.5))[0]

    n_pass = max(1, int(np.ceil(
        max(len(kp) for kp in keeps.values()) / (GH - 1))))

    bg32 = background.astype(np.float32) / np.float32(0.99)
    in_maps = []
    for p in range(n_pass):
        last = p == n_pass - 1
        maps = []
        for band in range(N_CORES):
            coef = np.zeros((6, GM), np.float32)
            thresh = np.full((GM, 1), 1e30, np.float32)
            cols = np.zeros((GM, 2 * C), np.float32)
            for xh in range(2):
                keep = keeps[(band, xh)][p * (GH - 1):(p + 1) * (GH - 1)]
                n = len(keep)
                s0 = xh * GH
                ka, kb, kc = a[keep], b[keep], c[keep]
                mx = m[keep, 0] - (xh * HALF_W + HALF_W / 2.0)
                my = m[keep, 1] - band * BAND_H - BAND_H / 2.0
                coef[0, s0:s0 + n] = ka
                coef[1, s0:s0 + n] = 2.0 * kb
                coef[2, s0:s0 + n] = kc
                coef[3, s0:s0 + n] = -2.0 * ka * mx - 2.0 * kb * my
                coef[4, s0:s0 + n] = -2.0 * kb * mx - 2.0 * kc * my
                coef[5, s0:s0 + n] = (ka * mx * mx + 2.0 * kb * mx * my
                                      + kc * my * my - 2.0 * ln_o[keep])
                thresh[s0:s0 + n, 0] = np.exp(
                    -0.5 * (tau[keep] - 2.0 * ln_o[keep])).astype(np.float32)
                cols[s0:s0 + n, xh * C:(xh + 1) * C] = col[keep]
                # background slot: alpha == 0.99, S == full log-transmittance
                thresh[s0 + GH - 1, 0] = 0.0
                coef[:, s0 + GH - 1] = 0.0
                cols[s0 + GH - 1] = 0.0
                if last:
                    cols[s0 + GH - 1, xh * C:(xh + 1) * C] = bg32
            maps.append({"coef": coef, "thresh": thresh, "cols": cols})
        in_maps.append(maps)
    return in_maps, n_pass


def _pixel_basis():
    ys, xs = np.meshgrid(
        np.arange(BAND_H, dtype=np.float32) - (BAND_H / 2.0 - 0.5),
        np.arange(HALF_W, dtype=np.float32) - (HALF_W / 2.0 - 0.5),
        indexing="ij")
    xs = xs.reshape(-1)
    ys = ys.reshape(-1)
    return np.stack([xs * xs, xs * ys, ys * ys, xs, ys,
                     np.ones_like(xs)], 0).astype(np.float32)


def _tri_blockdiag(np_s):
    tri = np.zeros((GM, GM), np.float32)
    blk = np.triu(np.ones((GH, GH), np.float32), 1)
    tri[:GH, :GH] = blk
    tri[GH:, GH:] = blk
    return tri.astype(np_s)


def kernel(means2d, conics, colors, opacities, depths, background,
           dt_q="float32", dt_s="float32", dt_img="bfloat16",
           _trace=False):
    import ml_dtypes
    from concourse.bass_utils import run_bass_kernel_spmd

    maps, n_pass = _host_prep(
        np.asarray(means2d), np.asarray(conics), np.asarray(colors),
        np.asarray(opacities), np.asarray(depths), np.asarray(background))
    nc = _build_program(dt_q, dt_s, dt_img, with_wlast=n_pass > 1)

    np_q = np.float32
    np_s = ml_dtypes.bfloat16 if dt_s == "bfloat16" else np.float32
    np_i = ml_dtypes.bfloat16 if dt_img == "bfloat16" else np.float32
    basis = _pixel_basis().astype(np_q)
    tri = _tri_blockdiag(np_s)

    acc = np.zeros((N_CORES, 2 * C, HPIX), np.float32)
    trans = np.ones((N_CORES, 2, 1, HPIX), np.float32)
    results = None
    for p in range(n_pass):
        in_maps = [{
            "basis": basis,
            "coef": maps[p][core]["coef"].astype(np_q),
            "tri": tri,
            "thresh": maps[p][core]["thresh"],
            "colors": maps[p][core]["cols"].astype(np_i),
        } for core in range(N_CORES)]
        results = run_bass_kernel_spmd(
            nc, in_maps, core_ids=list(range(N_CORES)), trace=_trace)
        for core in range(N_CORES):
            r = results.results[core]
            img = r["img"]
            for xh in range(2):
                acc[core, xh * C:(xh + 1) * C] += (
                    trans[core, xh] * img[xh * C:(xh + 1) * C])
                if n_pass > 1:
                    trans[core, xh] = trans[core, xh] * (
                        r["wlast"][xh:xh + 1].astype(np.float32)
                        / np.float32(0.99))

    out = np.empty((H, W_IMG, C), np.float32)
    for core in range(N_CORES):
        band = acc[core].reshape(2, C, BAND_H, HALF_W)
        r0 = core * BAND_H
        out[r0:r0 + BAND_H, :HALF_W] = band[0].transpose(1, 2, 0)
        out[r0:r0 + BAND_H, HALF_W:] = band[1].transpose(1, 2, 0)
    if _trace:
        return out, results
    return out
